# revision 25
# baseline (speedup 1.0000x reference)
"""Trainium2 Bass kernel for nn_ContrastiveLoss (SimCLR NT-Xent style loss).

Math (reference):
    reps = concat(zjs, zis)            # [8192, 128]
    rn = reps / ||reps||               # row-normalized
    sim = rn @ rn.T                    # [8192, 8192]
    per row i: pos = sim[i, i+-B]; den_i = sum_{j != i} exp(sim[i,j]/tau)
    CE = sum_i (log den_i - pos_i/tau);  pt = sum_i exp(pos_i/tau)/den_i
    loss = CE/N + B*(1/B - pt/(N*(N-1)))

Distribution: data-parallel over the 8192 rows, 1024 rows per NeuronCore.
Each core receives a column-ROTATED copy of rn^T (rolled by -1024*c, bf16,
normalized on the host) so the SPMD program is identical on every core.

Device work is the O(N^2) part only: the [1024, 8192] sim row-block (PE gram
in bf16) and the row sums of exp(10*sim) over all 8192 columns:
  - ACT: native exp + fused accumulator row sums on the first AW columns of
    each [128, 2048] PSUM tile
  - DVE: Schraudolph-style fast exp on the rest: the int16 affine
    sim*KSCH+CSCH is exactly the bf16 bit pattern of ~exp(10*sim); a bitcast
    bf16 row-reduce yields the partial sums
The host (fp64, O(N)) assembles den from the 64 chunk sums per row, subtracts
the self-similarity term exp(10*|rn_i|^2), computes pos from its own bf16 rn
replica (bit-identical to what the device multiplies), and the final scalar.
"""
import os

os.environ.setdefault("JAX_COMPILATION_CACHE_DIR", "/root/jax_bass_cache")

import math
import numpy as np
from contextlib import ExitStack

import concourse.bass as bass
import concourse.tile as tile
from concourse import mybir
from concourse.bass_utils import run_bass_kernel_spmd
from concourse.vector_clock import ScopedClock

# ---------------------------------------------------------------------------
# Workaround for walrus CoreV2/V3 "Too many sync wait commands": split sem
# waits so no instruction carries more than one, excess waits go onto
# preceding nofuse no-ops on the same engine.
# ---------------------------------------------------------------------------
_MAX_WAITS = int(os.environ.get("BASS_MAX_WAITS", "1"))
_orig_commit = tile.TileContext._commit_instruction


def _split_waits(nc, inst):
    si = getattr(inst, "sync_info", None)
    if si is None:
        return []
    waits = list(si.on_wait)
    if len(waits) <= _MAX_WAITS:
        return []
    nops = []
    excess, keep = waits[:-_MAX_WAITS], waits[-_MAX_WAITS:]
    for i in range(0, len(excess), _MAX_WAITS):
        nops.append(
            mybir.InstNoOp(
                name=nc.get_next_instruction_name(),
                engine=inst.engine,
                bass_nofuse=True,
                sync_info=mybir.SyncInfo(
                    on_wait=excess[i : i + _MAX_WAITS], on_update=[]
                ),
            )
        )
    inst.sync_info = mybir.SyncInfo(on_wait=keep, on_update=list(si.on_update))
    return nops


def _patched_commit(self, inst, lazy_reg_writes=True):
    try:
        nops = _split_waits(self.nc, inst)
    except Exception:
        nops = []
    for nop in nops:
        _orig_commit(self, nop)
    return _orig_commit(self, inst, lazy_reg_writes)


def _patched_drain_and_barrier(self, tick_clock, wait_clock):
    nc = self.nc
    probe = mybir.InstNoOp(
        name=nc.get_next_instruction_name(),
        engine=mybir.EngineType.SP,
        bass_nofuse=True,
    )
    wait_clock.add_sem_waits(probe, ScopedClock({None: tick_clock.global_clock}))
    si = probe.sync_info
    waits = list(si.on_wait) if si is not None else []
    for i in range(0, len(waits), _MAX_WAITS):
        nop = nc.sync.nop(nofuse=True)
        nop.ins.sync_info = mybir.SyncInfo(
            on_wait=waits[i : i + _MAX_WAITS], on_update=[]
        )
    nc.sync.drain()
    nc.all_engine_barrier()
    assert self.sems is not None
    popped = nc._tile_sem_poison_stack.pop()
    assert popped is self._sem_poison
    nc.clear_and_free_semaphores(list(self.sems.allocated().values()))
    nc.all_engine_barrier()


tile.TileContext._commit_instruction = _patched_commit
tile.TileContext._drain_and_barrier = _patched_drain_and_barrier

# ---------------------------------------------------------------------------
# Content-hashed NEFF cache: reuse a previously compiled NEFF when the BIR
# is byte-identical.
# ---------------------------------------------------------------------------
import hashlib
import shutil

_NEFF_CACHE_DIR = "/root/.bass_neff_cache"

import concourse.bass_utils as _bass_utils
import concourse.bass2jax as _bass2jax

_orig_compile_bir_kernel = _bass_utils.compile_bir_kernel


def _cached_compile_bir_kernel(bir_json, tmpdir, neff_name="file.neff"):
    try:
        key = hashlib.sha256(
            bir_json if isinstance(bir_json, bytes) else bir_json.encode()
        ).hexdigest()[:24]
        os.makedirs(_NEFF_CACHE_DIR, exist_ok=True)
        cached = os.path.join(_NEFF_CACHE_DIR, key + ".neff")
        if os.path.exists(cached):
            dst = os.path.join(tmpdir, neff_name)
            shutil.copy(cached, dst)
            return dst
    except Exception:
        cached = None
    neff_path = _orig_compile_bir_kernel(bir_json, tmpdir, neff_name)
    try:
        if cached:
            shutil.copy(neff_path, cached)
    except Exception:
        pass
    return neff_path


_bass_utils.compile_bir_kernel = _cached_compile_bir_kernel
_bass2jax.compile_bir_kernel = _cached_compile_bir_kernel

# ---------------------------------------------------------------------------
# Problem constants (hardcoded per contract)
# ---------------------------------------------------------------------------
B = 4096
N = 2 * B          # 8192 rows
D = 128            # feature dim
P = 128            # partitions
NCORES = 8
BLK = N // NCORES  # 1024 rows per core
NM = BLK // P      # 8 M-tiles per core
TAU = 0.1
SCALE = 1.0 / TAU  # 10.0

SIMW = 2048        # sim PSUM tile width (4 banks)
NSIM = N // SIMW   # 4 tiles per M row
QW = 512           # matmul moving width (one PSUM bank)

# Engine split of the exp over each [128, SIMW] sim tile
AW = 1472          # ACT columns (native exp + fused accum row sum)
SW = SIMW - AW     # Schraudolph columns (DVE affine + bitcast bf16 reduce)
KSCH = SCALE * 128.0 / math.log(2.0)    # sim -> bf16-exponent-field scale
CSCH = 127.0 * 128.0 - 7.5              # magic constant (tuned, round mode)

_cached_nc = None


def _build_nc():
    f32 = mybir.dt.float32
    bf16 = mybir.dt.bfloat16
    nc = bass.Bass()
    rnT = nc.declare_dram_parameter("rnT", [P, N], bf16, isOutput=False)
    rsout = nc.declare_dram_parameter("rsout", [P, NM * NSIM], f32, isOutput=True)
    rs2out = nc.declare_dram_parameter("rs2out", [P, NM * NSIM], f32, isOutput=True)

    with tile.TileContext(nc) as tc, ExitStack() as ctx:
        rnp = ctx.enter_context(tc.tile_pool(name="rnp", bufs=4))
        tailp = ctx.enter_context(tc.tile_pool(name="tailp", bufs=1))
        ep = ctx.enter_context(tc.tile_pool(name="ep", bufs=2))
        ebp = ctx.enter_context(tc.tile_pool(name="ebp", bufs=2))

        # rn slabs: [128, 2048] bf16 each, spread over two DMA queues
        # (not gpsimd: its DGE drain costs ~8us at context teardown)
        rn_tiles = []
        for s in range(NSIM):
            rn_t = rnp.tile([P, SIMW], bf16, tag="rn")
            eng = nc.sync if s % 2 == 0 else nc.scalar
            eng.dma_start(out=rn_t, in_=rnT[:, s * SIMW : (s + 1) * SIMW])
            rn_tiles.append(rn_t)

        rs_cols = tailp.tile([P, NM * NSIM], f32)
        rs2_cols = tailp.tile([P, NM * NSIM], f32)

        with tc.tile_pool(name="sim", bufs=2, space="PSUM") as simp:
            for nb in range(NSIM):
                for m in range(NM):
                    lhsT = rn_tiles[0][:, m * P : (m + 1) * P]
                    simt = simp.tile([P, SIMW], f32, tag="sim")
                    for q in range(SIMW // QW):
                        nc.tensor.matmul(
                            simt[:, q * QW : (q + 1) * QW], lhsT,
                            rn_tiles[nb][:, q * QW : (q + 1) * QW],
                            start=True, stop=True,
                        )
                    idx = m * NSIM + nb
                    # ACT: native exp + fused row-chunk sum on cols [0, AW)
                    e_t = ep.tile([P, AW], bf16, tag="e")
                    nc.scalar.activation(
                        out=e_t, in_=simt[:, 0:AW],
                        func=mybir.ActivationFunctionType.Exp,
                        scale=SCALE,
                        accum_out=rs_cols[:, idx : idx + 1],
                    )
                    # DVE: Schraudolph bits for cols [AW, SIMW): the int16
                    # affine sim*KSCH+CSCH is the bf16 pattern of exp(10*sim)
                    e_b = ebp.tile([P, SW], mybir.dt.int16, tag="eb")
                    nc.vector.tensor_scalar(
                        out=e_b, in0=simt[:, AW:SIMW],
                        scalar1=KSCH, scalar2=CSCH,
                        op0=mybir.AluOpType.mult, op1=mybir.AluOpType.add,
                    )
                    nc.vector.tensor_reduce(
                        out=rs2_cols[:, idx : idx + 1],
                        in_=e_b[:, :].bitcast(bf16),
                        axis=mybir.AxisListType.X, op=mybir.AluOpType.add,
                    )

            nc.sync.dma_start(out=rsout[:, :], in_=rs_cols)
            nc.sync.dma_start(out=rs2out[:, :], in_=rs2_cols)

    return nc


def _to_bf16_bits(x):
    """Round f32 array to bf16 (RNE), returning the bf16-valued f32 array."""
    u = np.ascontiguousarray(x, dtype=np.float32).view(np.uint32)
    rounded = (u + 0x7FFF + ((u >> 16) & 1)) & 0xFFFF0000
    return rounded.view(np.float32)


# Test/profiling hooks (unused by the grading path: TRACE defaults False).
TRACE = False
TRACE_DIR = None
LAST_RESULTS = None


def kernel(zis, zjs):
    global _cached_nc, LAST_RESULTS
    if _cached_nc is None:
        _cached_nc = _build_nc()
    nc = _cached_nc

    zis = np.asarray(zis, dtype=np.float32)
    zjs = np.asarray(zjs, dtype=np.float32)
    reps = np.concatenate([zjs, zis], axis=0)  # [8192, 128]

    # Host-side normalize (O(N*D), trivial next to the O(N^2) device work),
    # rounded to the exact bf16 values the device will multiply.
    norm = np.sqrt(np.sum(np.square(reps, dtype=np.float64), axis=1))
    rn = (reps / np.maximum(norm, 1e-8)[:, None]).astype(np.float32)
    rn_b = _to_bf16_bits(rn)  # f32 array holding bf16-rounded values

    in_maps = []
    for c in range(NCORES):
        rot = np.roll(rn_b, -BLK * c, axis=0)
        in_maps.append({"rnT": np.ascontiguousarray(rot.T).astype(
            __import__("ml_dtypes").bfloat16)})

    kwargs = {}
    if TRACE:
        kwargs = dict(trace=True, tmpdir=TRACE_DIR)
    res = run_bass_kernel_spmd(nc, in_maps, list(range(NCORES)), **kwargs)
    LAST_RESULTS = res

    # Host "all-reduce": den from the 64 chunk sums per row minus the self
    # term; pos from the bf16 rn replica; final scalar in fp64.
    rn64 = rn_b.astype(np.float64)
    selfsim = np.sum(rn64 * rn64, axis=1)              # [8192]
    pos_full = np.sum(rn64 * np.roll(rn64, -B, axis=0), axis=1)  # sim[i, i+B]

    den = np.empty(N, dtype=np.float64)
    for c, r in enumerate(res.results):
        rs = np.asarray(r["rsout"], np.float64)
        rs2 = np.asarray(r["rs2out"], np.float64)
        tot = (rs + rs2).reshape(P, NM, NSIM).sum(axis=2)  # [128, NM]
        den[c * BLK : (c + 1) * BLK] = tot.T.reshape(BLK)
    den -= np.exp(SCALE * selfsim)

    n = float(N)
    b = float(B)
    CE = float(np.sum(np.log(den) - SCALE * pos_full))
    pt = float(np.sum(np.exp(SCALE * pos_full) / den))
    loss = CE / n + b * (1.0 / b - pt / (n * (n - 1.0)))
    return np.float32(loss)
